# revision 58
# baseline (speedup 1.0000x reference)
"""Causal single-head attention (B=4, T=2048, D=1024, fp32) on 8 trn2 cores.

Sharding: each core takes one (batch, block-set) pair: batch b = core//2,
set s = core%2.  The 16 consecutive 128-row q-blocks of a batch are split
{0,3,4,7,8,11,12,15} / {1,2,5,6,9,10,13,14}: with a causal mask block g
only needs keys [0, 128*(g+1)), and both sets sum to 68 key-block units --
the optimal balanced split (vs 72 for a parity interleave).  The two sets
need different per-tile shapes, so two SPMD programs are built (one per
set), each run on 4 cores; they are symmetric in total work and simulate
to the same time.

Per q-tile pipeline (per core):
  S = Q_tile @ K^T (PE), computed as a 2-cycle/column hybrid instead of
     native fp32's 4 or the fp16 hi/lo 3-pass:
       4096*S ~= (64*qh)@(64*kh)          [fp16, 1 cyc/col/chunk]
               + e4m3(qh)@e4m3(4096*kl)   [fp8 DoubleRow, 0.25 cyc]
               + e4m3(4096*ql)@e4m3(kh)   [fp8 DoubleRow, 0.25 cyc]
     where qh=fp16(q), ql=q-qh (same for k).  The power-of-2 operand
     pre-scales make all three terms land at 4096*S so they accumulate
     into ONE fp32 PSUM tile with no merge pass.  Residual error is
     ~2e-4 rms on S (logit err ~6e-3 rms), far inside the 2e-2 gate.
  PSUM -> SBUF copy (ACT) with mask-bias add on the 128-wide diagonal
     block (DVE, from the real mask input), group-wise row maxes
     pipelined behind the matmuls (DVE).
  P = exp((32/4096)*S4096 - (32/4096)*max) (ACT, fp16 out, row-sums via
     accum_out)
  P^T per 128-block (PE transpose via identity, 8 blocks packed per
     PSUM bank), then fp8 P planes straight from the transpose bank:
     ph = e4m3(P^T) (ACT copy), pl = e4m3(P^T - ph) (DVE subtract).
  O += P^T.T @ V in fp8 DoubleRow pairs -- 3 passes per 256-deep pair
     (ph@vh + ph@vl + pl@vh at 0.5 cyc/col each vs fp16's 1 cyc per
     128), all landing at the same scale in one fp32 PSUM tile; odd
     tail blocks are padded with a zeroed lhs slot (GPSIMD memset).
     V's e4m3 hi/lo planes are split host-side.  d-major into separate
     PSUM tiles so each piece's scale+store overlaps the next piece's
     AV chain; the final tile splits its second half 384+128 so only a
     128-col scale+store remains in the kernel tail.
  O *= 1/rowsum per piece (ACT copy-with-scale; the final tile uses
     DVE for its first half), DMA out as fp16 (host upcasts).
The software pipeline is two tiles deep: while tile i runs QK on the
PE, tile i-1 runs exp/transpose/plane-derives (ACT/DVE) and tile i-2
runs its AV matmuls -- the extra stage gives the fp8 plane derivation
a whole QK slot of latency cover, so the PE never waits on it.  A run
of fill matmuls on a zeroed tile after q-tile 0 keeps the PE queue
from draining while the next tiles' operands stream in (a drain resets
the cost model's PE clock ramp and would double the next ~3us of
matmul costs).  K/V/Q loads are emitted in consumption order so the
serial DMA engines serve the next-needed operand; K fp16 is sliced in
512-col groups, the fp8 K hi-plane is DMA'd for the first 1024 cols
and derived from the fp16 slab (ACT copy at 2^-6) after that.

If the mask input is NOT exactly the causal triu mask, falls back to a
dense variant of the same program (all 16 key blocks per q-tile, full
mask bias applied) which is correct for any additive {0,1} mask.
"""

import numpy as np
import ml_dtypes

import concourse.mybir as mybir
import concourse.tile as tile
from concourse import bacc
from concourse.bass_utils import run_bass_kernel_spmd
from concourse.masks import make_identity

B, T, D = 4, 2048, 1024
NEG = -1000000000.0
P = 128          # partitions
NCORES = 8
NQT = 8          # q-tiles of 128 rows per core
CCHUNKS = D // P  # 8 contraction chunks
CPAIRS = CCHUNKS // 2  # 4 DoubleRow chunk pairs
STILES = T // P   # 16 key tiles per batch
F32 = mybir.dt.float32
F16 = mybir.dt.float16
FP8 = mybir.dt.float8e4
DR = mybir.MatmulPerfMode.DoubleRow
E4M3 = ml_dtypes.float8_e4m3

# Consecutive 128-row q-blocks per set; both sets sum to 68 units.
BLOCKS = {
    0: [0, 3, 4, 7, 8, 11, 12, 15],
    1: [1, 2, 5, 6, 9, 10, 13, 14],
}

# PSUM holds 4096*S; exp() folds the rescale into its input scale.
SEXP = 32.0 / 4096.0
WARM_N = 44  # PE fill matmuls after tile 0 (bridge the prologue DMA wait)
_cache = {}


def _blocks(causal: bool, par: int):
    if causal:
        return BLOCKS[par]
    return [8 * par + i for i in range(NQT)]


def _tile_cfg(causal: bool, par: int):
    """Per-q-tile (s_cols, bias_off, bias_cols), in execution order."""
    if causal:
        return [(P * (g + 1), P * g, P) for g in _blocks(causal, par)]
    return [(T, 0, T) for _ in range(NQT)]


def _build(causal: bool, par: int):
    cfg = _tile_cfg(causal, par)
    bias_cols = cfg[0][2]

    nc = bacc.Bacc("TRN2", target_bir_lowering=False, debug=False,
                   num_devices=4)
    # 64*fp16(q) pre-packed host-side in SBUF layout: [p, i, (c, j)]
    qhh = nc.declare_dram_parameter("qhh", [P, NQT, CCHUNKS * P], F16,
                                    isOutput=False)
    # fp8 q lo-plane pre-packed host-side in SBUF layout:
    # [p, i, (pair, t, j)] = e4m3(4096*ql).  The hi plane e4m3(qh) is
    # derived on ACT from the fp16 slab (one contiguous copy at 2^-6).
    q8d = nc.declare_dram_parameter("q8", [P, NQT, CPAIRS * 2 * P], FP8,
                                    isOutput=False)
    khh = nc.declare_dram_parameter("khh", [D, T], F16, isOutput=False)
    # fp8 k lo-plane pre-packed host-side: [p, pair, t, s].  The hi plane
    # e4m3(kh) is derived on-device from khh (ACT copy with scale 2^-6) --
    # except groups 0-1, whose hi planes are DMA'd so early tiles'
    # corrections don't wait on the ACT derive chain.
    k8d = nc.declare_dram_parameter("k8", [P, CPAIRS, 2, T], FP8,
                                    isOutput=False)
    k8h0 = nc.declare_dram_parameter("k8h0", [P, CPAIRS, 2, 1024], FP8,
                                     isOutput=False)
    # V fp8 hi/lo planes pre-packed host-side in DoubleRow pair layout
    # [p, pair, t, d] (s-tile = 2*pair + t): vh = e4m3(v), vl = e4m3(v-vh)
    v8hd = nc.declare_dram_parameter("v8h", [P, STILES // 2, 2, D], FP8,
                                     isOutput=False)
    v8ld = nc.declare_dram_parameter("v8l", [P, STILES // 2, 2, D], FP8,
                                     isOutput=False)
    # For the causal path the diagonal-block bias is identical for every
    # q-tile (block entry (j, u) is masked iff u > j), so a single
    # [P, 128] input suffices; the dense path keeps per-tile rows.
    if causal:
        biasd = nc.declare_dram_parameter("bias", [P, bias_cols], F32,
                                          isOutput=False)
    else:
        biasd = nc.declare_dram_parameter("bias", [NQT, P, bias_cols], F32,
                                          isOutput=False)
    out = nc.declare_dram_parameter("out", [NQT * P, D], F16, isOutput=True)

    AX = mybir.AxisListType.X
    EXP = mybir.ActivationFunctionType.Exp

    with tile.TileContext(nc) as tc:
        with (
            tc.tile_pool(name="const", bufs=1) as constp,
            tc.tile_pool(name="kv", bufs=1) as kvp,
            tc.tile_pool(name="qt", bufs=6) as qtp,
            tc.tile_pool(name="biasp", bufs=2) as biasp,
            tc.tile_pool(name="pp", bufs=3) as pp,
            tc.tile_pool(name="ssb", bufs=3) as ssbp,
            tc.tile_pool(name="php", bufs=4) as php,
            tc.tile_pool(name="plp", bufs=4) as plp,
            tc.tile_pool(name="outp", bufs=2) as outp,
            tc.tile_pool(name="stats", bufs=6) as statp,
            tc.tile_pool(name="ps_s", bufs=2, space="PSUM") as ps_sp,
            tc.tile_pool(name="ps_t", bufs=2, space="PSUM") as ps_tp,
            tc.tile_pool(name="ps_o", bufs=4, space="PSUM") as ps_op,
        ):
            ident = constp.tile([P, P], F16)
            warm = constp.tile([P, 256], F16, name="warm")
            nc.gpsimd.memset(warm[:], 0.0)
            bias_res = None
            if causal:
                bias_res = constp.tile([P, bias_cols], F32, name="bias_res")

            # K^T / V stay SBUF-resident; their loads are emitted inside the
            # q-tile loop in consumption order so q-tile 0's operands aren't
            # queued behind 16MB of K/V DMA.
            # kh16: one slab [P, CCHUNKS*T]; chunk c occupies cols [c*T,(c+1)*T)
            kt16 = kvp.tile([P, CCHUNKS * T], F16, name="kt16")
            # fp8 planes, one slab: [P, plane(2), pair(4), t(2), T] --
            # plane-major so (pair, t) merge into one stride-T dim for DMA
            k8s = kvp.tile([P, 2 * CPAIRS * 2 * T], FP8, name="k8s")
            k8v = k8s.rearrange("p (h r t s) -> p h r t s", h=2, r=CPAIRS,
                                t=2)
            # V fp8 plane slabs [P, pair, t, D], DoubleRow pair layout
            v8h_sl = kvp.tile([P, STILES * D], FP8, name="v8h_sl")
            v8h = v8h_sl.rearrange("p (r t d) -> p r t d", r=STILES // 2,
                                   t=2)
            v8l_sl = kvp.tile([P, STILES * D], FP8, name="v8l_sl")
            v8l = v8l_sl.rearrange("p (r t d) -> p r t d", r=STILES // 2,
                                   t=2)
            # identity for the P^T transposes
            make_identity(nc, ident[:])

            kt_loaded = 0  # next 512-col chunk of kT to load
            v_loaded = 0   # next 4-tile group of V to load
            bias_loaded = False
            nkt_max = max((sc + 511) // 512 for sc, _, _ in cfg)

            state = {}      # q-tile -> tensors produced by compute_a
            dma_state = {}  # q-tile -> qt tiles in flight
            pending_out = []  # deferred (tile, o_sb) stores

            def dma_qt(i):
                """Q slab DMAs for q-tile i (prefetched tiles ahead)."""
                qt16 = qtp.tile([P, CCHUNKS * P], F16, tag="qt16",
                                name="qt16")
                nc.sync.dma_start(qt16[:], qhh[:, i, :])
                # fp8 q slab: [P, plane(2), pair(4), t(2), j(128)] --
                # plane-major so the lo-plane DMA and the hi-plane derive
                # are both single contiguous ops
                qt8 = qtp.tile([P, 2 * CPAIRS * 2 * P], FP8, tag="qt8",
                               name="qt8")
                nc.sync.dma_start(qt8[:, CPAIRS * 2 * P:], q8d[:, i, :])
                dma_state[i] = [qt16, qt8, None]

            def load_kt_group():
                nonlocal kt_loaded
                g = kt_loaded
                sl = slice(g * 512, (g + 1) * 512)
                nc.sync.dma_start(
                    kt16.rearrange("p (c s) -> p c s", s=T)[:, :, sl],
                    khh[:, sl].rearrange("(c p) s -> p c s", p=P))
                nc.sync.dma_start(k8v[:, 1, :, :, sl], k8d[:, :, :, sl])
                if g <= 1:
                    nc.sync.dma_start(k8v[:, 0, :, :, sl],
                                      k8h0[:, :, :, g * 512:(g + 1) * 512])
                else:
                    # derive the hi plane e4m3(kh) from 64*fp16(kh) in
                    # one wide ACT op across all pairs
                    nc.scalar.mul(
                        k8v[:, 0, :, :, sl],
                        kt16.rearrange("p (r t s) -> p r t s", r=CPAIRS,
                                       t=2)[:, :, :, sl],
                        1.0 / 64.0)
                kt_loaded += 1

            def dma_k(i):
                """K chunk DMAs (+ bias, first time) required by q-tile i,
                plus the tile's fp8 q hi-plane derive."""
                s_cols, b_off, b_cols = cfg[i]
                qt16_i, qt8_i = dma_state[i][0], dma_state[i][1]
                nc.scalar.mul(qt8_i[:, :CPAIRS * 2 * P], qt16_i[:],
                              1.0 / 64.0)
                want_kt = (s_cols + 511) // 512
                while kt_loaded < want_kt:
                    load_kt_group()
                if causal:
                    nonlocal bias_loaded
                    if not bias_loaded:
                        nc.sync.dma_start(bias_res[:], biasd[:])
                        bias_loaded = True
                    bias_sb = bias_res
                else:
                    bias_sb = biasp.tile([P, b_cols], F32, tag="bias",
                                         name="bias_sb")
                    nc.sync.dma_start(bias_sb[:], biasd[i])
                dma_state[i][2] = bias_sb

            def dma_v(i):
                """V fp8 plane DMAs (2 pairs = 4 s-tiles per op) first
                used by q-tile i's stage B, then one K group of
                lookahead."""
                s_cols = cfg[i][0]
                nonlocal v_loaded
                want_v = (min(s_cols // P, STILES) + 3) // 4 if causal \
                    else (STILES + 3) // 4
                while v_loaded < want_v:
                    g = v_loaded
                    nc.sync.dma_start(v8h[:, 2 * g:2 * g + 2],
                                      v8hd[:, 2 * g:2 * g + 2])
                    nc.sync.dma_start(v8l[:, 2 * g:2 * g + 2],
                                      v8ld[:, 2 * g:2 * g + 2])
                    v_loaded += 1
                if kt_loaded < nkt_max:
                    load_kt_group()

            def compute_a(i):
                """QK matmuls into per-group PSUM, copy to SBUF S, mask
                bias add, row-max stats."""
                s_cols, b_off, b_cols = cfg[i]
                ngroups = (s_cols + 511) // 512
                qt16, qt8, bias_sb = dma_state.pop(i)
                qt8v = qt8.rearrange("p (h r t j) -> p h r t j", h=2,
                                     r=CPAIRS, t=2)
                kt16v = kt16.rearrange("p (c s) -> p c s", s=T)

                s_sb = ssbp.tile([P, s_cols], F32, tag="s_sb", name="s_sb")
                pmax = statp.tile([P, ngroups], F32, tag="pmax", name="pmax")
                for g in range(ngroups):
                    g0 = g * 512
                    gw = min(512, s_cols - g0)
                    sl = slice(g0, g0 + gw)
                    ps = ps_sp.tile([P, 512], F32, tag="s", name="ps_g")
                    for c in range(CCHUNKS):
                        nc.tensor.matmul(
                            ps[:, :gw],
                            qt16[:, c * P:(c + 1) * P],
                            kt16v[:, c, sl],
                            start=(c == 0), stop=False)
                    for cp in range(CPAIRS):
                        # qh8 @ kl8' :  plane q0 x plane k1
                        nc.tensor.matmul(
                            ps[:, :gw],
                            qt8v[:, 0, cp], k8v[:, 1, cp, :, sl],
                            start=False, stop=False, perf_mode=DR)
                        # ql8' @ kh8 :  plane q1 x plane k0
                        nc.tensor.matmul(
                            ps[:, :gw],
                            qt8v[:, 1, cp], k8v[:, 0, cp, :, sl],
                            start=False, stop=(cp == CPAIRS - 1),
                            perf_mode=DR)
                    # PSUM -> SBUF: plain copy outside the mask band (ACT),
                    # fused bias-add inside it (DVE).
                    lo = max(g0, b_off)
                    hi = min(g0 + gw, b_off + b_cols)
                    if lo < hi:
                        if lo > g0:
                            nc.scalar.copy(s_sb[:, g0:lo], ps[:, :lo - g0])
                        nc.vector.tensor_add(
                            s_sb[:, lo:hi], ps[:, lo - g0:hi - g0],
                            bias_sb[:, lo - b_off:hi - b_off])
                        if hi < g0 + gw:
                            nc.scalar.copy(s_sb[:, hi:g0 + gw],
                                           ps[:, hi - g0:gw])
                    else:
                        nc.scalar.copy(s_sb[:, g0:g0 + gw], ps[:, :gw])
                    nc.vector.reduce_max(pmax[:, g:g + 1], s_sb[:, g0:g0 + gw],
                                         axis=AX)
                negm = statp.tile([P, 1], F32, tag="negm", name="negm")
                nc.vector.reduce_max(negm[:], pmax[:, :ngroups], axis=AX,
                                     negate=True)
                negms = statp.tile([P, 1], F32, tag="negms", name="negms")
                nc.vector.tensor_scalar_mul(negms[:], negm[:], SEXP)
                state[i] = (s_sb, negms)

            state2 = {}  # q-tile -> (ph slabs, pl slabs, rinv)

            def stage_b1(i):
                """exp + row-sum, P^T transposes, and the fp8 P-plane
                derives: ph = e4m3(pT) on ACT, pl = e4m3(pT - ph) on
                DVE, straight from the transpose PSUM bank."""
                s_cols, _, _ = cfg[i]
                stiles = s_cols // P
                ngroups = (s_cols + 511) // 512
                s_sb, negms = state.pop(i)

                p_sb = pp.tile([P, s_cols], F16, tag="p", name="p_sb")
                gsum = statp.tile([P, ngroups], F32, tag="gsum", name="gsum")
                for g in range(ngroups):
                    g0 = g * 512
                    gw = min(512, s_cols - g0)
                    nc.scalar.activation(
                        p_sb[:, g0:g0 + gw], s_sb[:, g0:g0 + gw], EXP,
                        bias=negms[:], scale=SEXP,
                        accum_out=gsum[:, g:g + 1])
                rsum = statp.tile([P, 1], F32, tag="rsum", name="rsum")
                nc.vector.reduce_sum(rsum[:], gsum[:, :ngroups], axis=AX)
                rinv = statp.tile([P, 1], F32, tag="rinv", name="rinv")
                nc.vector.reciprocal(rinv[:], rsum[:])

                phs, pls = [], []
                for st0 in range(0, stiles, 8):
                    nblk = min(8, stiles - st0)
                    ps_t = ps_tp.tile([P, 8 * P], F16, tag="t", name="ps_t")
                    for j in range(nblk):
                        st = st0 + j
                        nc.tensor.transpose(ps_t[:, j * P:(j + 1) * P],
                                            p_sb[:, st * P:(st + 1) * P],
                                            ident[:])
                    ph = php.tile([P, 8 * P], FP8, tag="ph", name="ph")
                    pl = plp.tile([P, 8 * P], FP8, tag="pl", name="pl")
                    nc.scalar.mul(ph[:, :nblk * P], ps_t[:, :nblk * P], 1.0)
                    nc.vector.tensor_sub(pl[:, :nblk * P],
                                         ps_t[:, :nblk * P],
                                         ph[:, :nblk * P])
                    if nblk % 2:
                        # zero the DoubleRow pad slot next to the odd
                        # last block (GPSIMD; off every critical path)
                        nc.gpsimd.memset(ph[:, nblk * P:(nblk + 1) * P], 0.0)
                        nc.gpsimd.memset(pl[:, nblk * P:(nblk + 1) * P], 0.0)
                    phs.append(ph)
                    pls.append(pl)
                state2[i] = (phs, pls, rinv)

            def stage_b2(i):
                """AV accumulation in fp8 DoubleRow pairs (3 passes:
                ph@vh + ph@vl + pl@vh, all landing at 16*O in one PSUM
                tile), 1/sum scale, output DMA."""
                s_cols, _, _ = cfg[i]
                stiles = s_cols // P
                npairs = (stiles + 1) // 2
                phs, pls, rinv = state2.pop(i)

                final = i == NQT - 1
                # d-slices with SEPARATE PSUM tiles, so each piece's
                # scale + store overlaps the next piece's AV chain; the
                # final tile splits the second half so only a 128-col
                # scale+store remains in the kernel tail
                dsl = [(0, 512), (512, 896), (896, 1024)] if final \
                    else [(0, 512), (512, 1024)]
                o_sb = outp.tile([P, D], F16, tag="o_sb", name="o_sb")
                for pi, (d0, d1) in enumerate(dsl):
                    ps_o = ps_op.tile([P, d1 - d0], F32, tag="o",
                                      name="ps_o")
                    for pr in range(npairs):
                        lph = phs[pr // 4][:, (pr % 4) * 256:
                                           (pr % 4 + 1) * 256]
                        lph = lph.rearrange("p (t j) -> p t j", t=2)
                        lpl = pls[pr // 4][:, (pr % 4) * 256:
                                           (pr % 4 + 1) * 256]
                        lpl = lpl.rearrange("p (t j) -> p t j", t=2)
                        nc.tensor.matmul(
                            ps_o[:], lph, v8h[:, pr, :, d0:d1],
                            start=(pr == 0), stop=False, perf_mode=DR)
                        nc.tensor.matmul(
                            ps_o[:], lph, v8l[:, pr, :, d0:d1],
                            start=False, stop=False, perf_mode=DR)
                        nc.tensor.matmul(
                            ps_o[:], lpl, v8h[:, pr, :, d0:d1],
                            start=False, stop=(pr == npairs - 1),
                            perf_mode=DR)
                    if final:
                        if pi == 0:
                            # DVE so the scale (and its store's descriptor
                            # latency) runs under the later AV chains
                            nc.vector.tensor_scalar_mul(o_sb[:, d0:d1],
                                                        ps_o[:], rinv[:])
                        else:
                            nc.scalar.mul(o_sb[:, d0:d1], ps_o[:], rinv[:])
                        nc.sync.dma_start(out[i * P:(i + 1) * P, d0:d1],
                                          o_sb[:, d0:d1])
                    else:
                        nc.scalar.mul(o_sb[:, d0:d1], ps_o[:], rinv[:])
                if not final:
                    pending_out.append((i, o_sb))

            # Software pipeline: QK of one tile runs (on PE) while the
            # previous tile does softmax/exp on ACT/DVE.
            dma_qt(0)
            dma_k(0)
            dma_qt(1)
            dma_qt(2)
            dma_v(0)
            dma_qt(3)
            dma_qt(4)
            for idx in range(NQT + 2):
                if idx < NQT:
                    if idx + 1 < NQT:
                        dma_k(idx + 1)
                        dma_v(idx + 1)
                    if idx + 5 < NQT:
                        dma_qt(idx + 5)
                    while pending_out:
                        oi, osb = pending_out.pop(0)
                        nc.sync.dma_start(out[oi * P:(oi + 1) * P, :],
                                          osb[:])
                    compute_a(idx)
                    if idx == 0:
                        # fill matmuls on a zeroed tile: they bridge the
                        # PE-idle window while the next tiles' operands
                        # stream in, so the PE queue never drains (a
                        # drain resets the cost model's clock ramp and
                        # doubles the next ~3us of matmul costs)
                        for w in range(WARM_N):
                            # transpose-bank pool: idle in the prologue,
                            # so fills never rotate against QK groups
                            ps_w = ps_tp.tile([P, 256], F32, tag="t",
                                              name="ps_w")
                            nc.tensor.matmul(ps_w[:], warm[:, :P],
                                             warm[:], start=True, stop=True)
                if 1 <= idx <= NQT:
                    stage_b1(idx - 1)
                if idx >= 2:
                    stage_b2(idx - 2)
            while pending_out:
                oi, osb = pending_out.pop(0)
                nc.sync.dma_start(out[oi * P:(oi + 1) * P, :], osb[:])

    nc.compile()
    return nc


def _rows(causal: bool, par: int) -> np.ndarray:
    return np.concatenate(
        [P * g + np.arange(P) for g in _blocks(causal, par)])


def _get(causal: bool, par: int):
    key = (causal, par)
    if key not in _cache:
        _cache[key] = _build(causal, par)
    return _cache[key]


def _split_hl(x):
    """fp32 [n, d] -> (fp16 hi, fp32 lo residual)."""
    hi = x.astype(np.float16)
    lo = x - hi.astype(np.float32)
    return hi, lo


def kernel(query, key, value, mask):
    query = np.asarray(query, dtype=np.float32)
    key = np.asarray(key, dtype=np.float32)
    value = np.asarray(value, dtype=np.float32)
    mask = np.asarray(mask, dtype=np.float32)

    causal = bool(
        np.array_equal(mask, np.triu(np.ones((T, T), np.float32), k=1)))
    # bias folded pre-scale: SEXP*(S4096 + mask*NEG/SEXP) == SEXP*S4096
    # + mask*NEG exactly (NEG/SEXP = NEG*128)
    mask_scaled = mask * np.float32(NEG / SEXP)

    # per batch: kh/kl planes, fp16*64 and fp8
    khh_b, k8_b, v_b = [], [], []
    for b in range(B):
        kT = np.ascontiguousarray(key[b].T)  # [D, T]
        kh, kl = _split_hl(kT)
        khh_b.append(np.ascontiguousarray(
            (kh.astype(np.float32) * 64.0).astype(np.float16)))
        # lo plane [D, T] -> [p, pair, t, s]; hi plane for cols < 1024
        k8 = (kl * 4096.0).astype(E4M3)
        k8 = k8.reshape(CPAIRS, 2, P, T).transpose(2, 0, 1, 3)
        k8h = kh.astype(np.float32)[:, :1024].astype(E4M3)
        k8h = k8h.reshape(CPAIRS, 2, P, 1024).transpose(2, 0, 1, 3)
        k8_b.append((np.ascontiguousarray(k8), np.ascontiguousarray(k8h)))
        # V fp8 hi/lo planes in DoubleRow pair layout [p, pair, t, d]
        vb = value[b]
        vh = vb.astype(E4M3)
        vl = (vb - vh.astype(np.float32)).astype(E4M3)
        v_b.append(tuple(
            np.ascontiguousarray(
                x.reshape(STILES // 2, 2, P, D).transpose(2, 0, 1, 3))
            for x in (vh, vl)))

    in_maps = {0: [], 1: []}
    rows_by_core = []
    for c in range(NCORES):
        b, par = c // 2, c % 2
        cfg = _tile_cfg(causal, par)
        rows = _rows(causal, par)
        rows_by_core.append((b, par, rows))
        qT_c = np.ascontiguousarray(query[b][rows].T)  # [D, rows]
        qh, ql = _split_hl(qT_c)
        if causal:
            _, boff, bcols = cfg[0]
            # diagonal block bias: identical for every q-tile
            bias_c = mask_scaled[0:P, 0:bcols]
        else:
            bias_c = np.stack([
                mask_scaled[rows[i * P:(i + 1) * P], boff:boff + bcols]
                for i, (_, boff, bcols) in enumerate(cfg)])
        # lo plane [D, n] -> [p, i, pair, t, j] -> [p, i, flat]
        q8 = (ql * 4096.0).astype(E4M3)
        q8 = q8.reshape(CPAIRS, 2, P, NQT, P).transpose(2, 3, 0, 1, 4)
        q8 = q8.reshape(P, NQT, CPAIRS * 2 * P)
        # fp16 hi: [D, n] -> [c, p, i, j] -> [p, i, (c j)]
        q16 = (qh.astype(np.float32) * 64.0).astype(np.float16)
        q16 = q16.reshape(CCHUNKS, P, NQT, P).transpose(1, 2, 0, 3)
        q16 = q16.reshape(P, NQT, CCHUNKS * P)
        im = {
            "qhh": np.ascontiguousarray(q16),
            "q8": np.ascontiguousarray(q8),
            "khh": khh_b[b],
            "k8": k8_b[b][0],
            "k8h0": k8_b[b][1],
            "v8h": v_b[b][0],
            "v8l": v_b[b][1],
            "bias": np.ascontiguousarray(bias_c),
        }
        in_maps[par].append(im)

    res = {}
    for par in (0, 1):
        nc = _get(causal, par)
        res[par] = run_bass_kernel_spmd(nc, in_maps[par],
                                        core_ids=[0, 1, 2, 3])

    outp = np.empty((B, T, D), dtype=np.float32)
    for c in range(NCORES):
        b, par, rows = rows_by_core[c]
        outp[b][rows] = res[par].results[b]["out"].astype(np.float32)
    return outp
